# revision 2
# baseline (speedup 1.0000x reference)
"""Trainium2 Bass kernel for BlockPrototypeMemory (sparse block attention), v2.

Reference computation (fp32):
  mem = 4-layer MLP(mem_params)            [1, P, NB, DB]
  mem = block_ln(mem); q = block_ln(queries)
  scores = einsum('bnhd,zmhd->bhnm', q*DB^-.5, mem)
  out = softmax(scores) @ mem              [B, N, D]

Sharding: tensor-parallel over the NB=16 blocks; each of the 8 cores owns
2 blocks (a 256-wide slice of D).  No collectives: the host reassembles.

v2 design (vs the 638us baseline, which stalled ~50% on PSUM-drain deps):
 - QK^T in fp8e4 DoubleRow (2 k-tiles of 64 along DB, even/odd d pairs
   packed in bytes so the 2-byte DMA transpose carries fp8 pairs).
   sqrt(DB^-0.5) is folded into each side (q via istd, k at cast).
 - exp on ACT straight out of PSUM -> E in fp8e5 (bias -1 folded into the
   softmax ratio); PV in DoubleRow fp8 (E e5m2 x kv e4m3) with a ones
   column accumulating the denominator.
 - PV accumulates into the *same* PSUM tile the scores came from (banks are
   dead after exp), so two 4-bank tiles give a 2-deep pipeline and the PE
   never waits on PSUM for more than one drain.
 - q LN stats via per-(subtile, block) single-group bn_stats on DVE
   (even/odd halves recombined with 4 vector ops); mean subtraction of q
   is dropped (keys are zero-mean).
 - Output stored bf16 in tile-dump layout [bt, p, s, h, d]; host transposes
   and converts (device normalizes by the denominator; host only relayouts).
Emission order per step interleaves next-batch loads/stats so the ACT
engine (exp, ~1.85us/unit) streams with no batch-boundary bubble.
"""

import numpy as np
from ml_dtypes import bfloat16

import concourse.bass as bass
import concourse.bacc as bacc
import concourse.mybir as mybir
import concourse.tile as tile
from concourse.bass import ts
from concourse.bass_utils import run_bass_kernel_spmd

F32 = mybir.dt.float32
BF16 = mybir.dt.bfloat16
FP8E4 = mybir.dt.float8e4
FP8E5 = mybir.dt.float8e5
I32 = mybir.dt.int32
ALU = mybir.AluOpType
ACT = mybir.ActivationFunctionType
AX = mybir.AxisListType
DR = mybir.MatmulPerfMode.DoubleRow

# ---- problem dims ----
B, N, D = 8, 4096, 2048
NB, DB = 16, 128
P = 512            # prototypes (attention keys)
HID = 4 * DB       # 512 MLP hidden
NCORES = 8
HPC = NB // NCORES  # 2 blocks per core
DS = HPC * DB       # 256 per-core D slice
EPS = 1e-5
KC = P // 128       # 4 key chunks
JC = HID // 128     # 4 hidden chunks
SQ = float(DB) ** -0.25   # sqrt of the DB^-0.5 score scale, one per side
EBIAS = -1.0              # exp(s + EBIAS): cancels in the softmax ratio


def _rsqrt(nc, pool, x, out, F, tag):
    """out = 1/sqrt(x) elementwise on DVE only (no ACT table swaps)."""
    ti = pool.tile([128, F], I32, tag=tag + "_i")
    t2 = pool.tile([128, F], F32, tag=tag + "_t")
    y = pool.tile([128, F], F32, tag=tag + "_y")
    magic = pool.tile([128, 1], I32, tag=tag + "_m")
    nc.vector.memset(magic[:], 0x5F3759DF)
    nc.vector.tensor_scalar(ti[:], x.bitcast(I32), 1, None, op0=ALU.arith_shift_right)
    nc.vector.tensor_tensor(ti[:], magic[:].broadcast_to([128, F]), ti[:], ALU.subtract)
    yv = ti[:].bitcast(F32)
    for it in range(3):
        dst = out if it == 2 else y[:]
        nc.vector.tensor_tensor(t2[:], yv, yv, ALU.mult)
        nc.vector.tensor_tensor(t2[:], t2[:], x, ALU.mult)
        nc.vector.tensor_scalar(t2[:], t2[:], -0.5, 1.5, op0=ALU.mult, op1=ALU.add)
        nc.vector.tensor_tensor(dst, yv, t2[:], ALU.mult)
        yv = y[:]


def build_nc(nbb=B, nt=N // 512):
    """Per-core Bass module. nbb batches x nt 512-token groups."""
    nc = bacc.Bacc("TRN2", target_bir_lowering=False, debug=False)
    rows = nbb * nt * 512
    nsub = 4 * nt           # 128-token subtiles per batch

    q_d = nc.dram_tensor("q", [rows, DS], F32, kind="ExternalInput").ap()
    mpt_d = nc.dram_tensor("mpt", [HPC, DB, P], F32, kind="ExternalInput").ap()
    w1t_d = nc.dram_tensor("w1t", [DB, HID], BF16, kind="ExternalInput").ap()
    w2t_d = nc.dram_tensor("w2t", [HID, HID], BF16, kind="ExternalInput").ap()
    w3t_d = nc.dram_tensor("w3t", [HID, HID], BF16, kind="ExternalInput").ap()
    w4t_d = nc.dram_tensor("w4t", [HID, DB], BF16, kind="ExternalInput").ap()
    b1r_d = nc.dram_tensor("b1r", [DB, JC], F32, kind="ExternalInput").ap()
    b2r_d = nc.dram_tensor("b2r", [DB, JC], F32, kind="ExternalInput").ap()
    b3r_d = nc.dram_tensor("b3r", [DB, JC], F32, kind="ExternalInput").ap()
    b4rep_d = nc.dram_tensor("b4rep", [DB, DB], F32, kind="ExternalInput").ap()
    # tile-dump layout: [b*nt + t, p, s, h, d+den] bf16; host reassembles
    out_d = nc.dram_tensor("out", [nbb * nt, 128, 4, HPC, DB + 1], BF16,
                           kind="ExternalOutput").ap()

    q_v = q_d.rearrange("(b s p) d -> b p s d", b=nbb, p=128)

    with tile.TileContext(nc) as tc:
        with (
            tc.tile_pool(name="const", bufs=1) as const,
            tc.tile_pool(name="qres", bufs=3) as qres_p,
            tc.tile_pool(name="stat", bufs=3) as stat_p,
            tc.tile_pool(name="qb", bufs=12) as qb_p,
            tc.tile_pool(name="qT", bufs=10) as qT_p,
            tc.tile_pool(name="E", bufs=4) as e_p,
            tc.tile_pool(name="ob", bufs=6) as ob_p,
        ):
            # ---- persistent small tensors ----
            ebias = const.tile([128, 1], F32)
            nc.vector.memset(ebias[:], EBIAS)
            # PV keys: LN'd mem (unscaled) fp8e4 + ones col for denominator
            kv8 = const.tile([128, HPC, KC, 130], FP8E4)
            nc.vector.memset(kv8[:, :, :, 128:130], 0.0)
            nc.vector.memset(kv8[:, :, :, 128:129], 1.0)
            # QK keys: LN'd mem * SQ, transposed+packed (even/odd d pairs)
            kT8 = const.tile([128, KC, 128, 2], FP8E4)
            # j-major copy: dual-fp8 Ldweights requires contiguous k-tiles
            kT8u = const.tile([128, KC, 2, 128], FP8E4)

            # ---- batch-0 loads first: DMA runs during the MLP phase ----
            state = {}

            def emit_loads(b, lo=0, hi=None):
                nchunk = min(8, nsub)
                if lo == 0:
                    state[b] = {"qr": qres_p.tile([128, nsub, DS], F32,
                                                  tag="qr", name=f"qr{b}")}
                qr = state[b]["qr"]
                w = nsub // nchunk
                if hi is None:
                    hi = nchunk
                for k in range(lo, hi):
                    nc.sync.dma_start(qr[:, k * w:(k + 1) * w, :],
                                      q_v[b, :, k * w:(k + 1) * w, :])

            emit_loads(0)
            if nbb > 1:
                emit_loads(1)

            # ---- phase 0: mem MLP + LN per block ----
            with (
                tc.tile_pool(name="mlp_ps", bufs=2, space="PSUM") as mlp_ps,
                tc.tile_pool(name="mlp_ps4", bufs=2, space="PSUM") as mlp_ps4,
                tc.tile_pool(name="mlp_sb", bufs=1) as mlp_sb,
            ):
                b1r = const.tile([DB, JC], F32)
                b2r = const.tile([DB, JC], F32)
                b3r = const.tile([DB, JC], F32)
                b4rep = const.tile([DB, DB], F32)
                w1t_b = mlp_sb.tile([DB, HID], BF16, tag="w1b")
                w2t_b = mlp_sb.tile([128, JC, HID], BF16, tag="w2b")
                w3t_b = mlp_sb.tile([128, JC, HID], BF16, tag="w3b")
                w4t_b = mlp_sb.tile([128, JC, DB], BF16, tag="w4b")
                nc.sync.dma_start(w1t_b[:], w1t_d)
                nc.sync.dma_start(w2t_b[:], w2t_d.rearrange("(c p) o -> p c o", p=128))
                nc.sync.dma_start(w3t_b[:], w3t_d.rearrange("(c p) o -> p c o", p=128))
                nc.sync.dma_start(w4t_b[:], w4t_d.rearrange("(c p) o -> p c o", p=128))
                nc.sync.dma_start(b1r[:], b1r_d)
                nc.sync.dma_start(b2r[:], b2r_d)
                nc.sync.dma_start(b3r[:], b3r_d)
                nc.sync.dma_start(b4rep[:], b4rep_d)

                # packed pre-transpose keys: [p(protos), r, h, d] fp8e4
                kvs8 = mlp_sb.tile([128, KC, HPC, DB], FP8E4, tag="kvs8")
                for h in range(HPC):
                    x_f = mlp_sb.tile([DB, P], F32, tag="xf")
                    x_b = mlp_sb.tile([DB, P], BF16, tag="xb")
                    nc.sync.dma_start(x_f[:], mpt_d[h])
                    nc.vector.tensor_copy(x_b[:], x_f[:])
                    h1 = mlp_sb.tile([128, JC, P], BF16, tag="h1")
                    for j in range(JC):
                        ps = mlp_ps.tile([128, P], F32, tag="ps")
                        nc.tensor.matmul(ps[:], w1t_b[:, ts(j, 128)], x_b[:],
                                         start=True, stop=True)
                        nc.scalar.activation(h1[:, j, :], ps[:], ACT.Relu,
                                             bias=b1r[:, j:j + 1])
                    h2 = mlp_sb.tile([128, JC, P], BF16, tag="h2")
                    for j in range(JC):
                        ps = mlp_ps.tile([128, P], F32, tag="ps")
                        for i in range(JC):
                            nc.tensor.matmul(ps[:], w2t_b[:, i, ts(j, 128)],
                                             h1[:, i, :],
                                             start=(i == 0), stop=(i == JC - 1))
                        nc.scalar.activation(h2[:, j, :], ps[:], ACT.Relu,
                                             bias=b2r[:, j:j + 1])
                    h3 = mlp_sb.tile([128, JC, P], BF16, tag="h3")
                    for j in range(JC):
                        ps = mlp_ps.tile([128, P], F32, tag="ps")
                        for i in range(JC):
                            nc.tensor.matmul(ps[:], w3t_b[:, i, ts(j, 128)],
                                             h2[:, i, :],
                                             start=(i == 0), stop=(i == JC - 1))
                        nc.scalar.activation(h3[:, j, :], ps[:], ACT.Relu,
                                             bias=b3r[:, j:j + 1])
                    # L4 in row layout [keys, DB] so LN stats are free-dim
                    m_f = mlp_sb.tile([128, KC, DB], F32, tag="mf")
                    for r in range(KC):
                        ps4 = mlp_ps4.tile([128, DB], F32, tag="ps4")
                        for i in range(JC):
                            nc.tensor.matmul(ps4[:], h3[:, i, ts(r, 128)],
                                             w4t_b[:, i, :],
                                             start=(i == 0), stop=(i == JC - 1))
                        nc.vector.tensor_tensor(m_f[:, r, :], ps4[:], b4rep[:],
                                                ALU.add)
                    # block-LN over DB (free dim) for the 4 row chunks
                    msum = mlp_sb.tile([128, KC], F32, tag="msum")
                    msq = mlp_sb.tile([128, KC], F32, tag="msq")
                    scr = mlp_sb.tile([128, KC, DB], F32, tag="scr")
                    nc.vector.reduce_sum(msum[:], m_f[:], axis=AX.X)
                    nc.scalar.activation(scr[:], m_f[:], ACT.Square)
                    nc.vector.reduce_sum(msq[:], scr[:], axis=AX.X)
                    mu = mlp_sb.tile([128, KC], F32, tag="mu")
                    var = mlp_sb.tile([128, KC], F32, tag="var")
                    nc.vector.tensor_scalar(mu[:], msum[:], 1.0 / DB, None, op0=ALU.mult)
                    nc.vector.tensor_scalar(var[:], msq[:], 1.0 / DB, None, op0=ALU.mult)
                    nc.vector.tensor_tensor(scr[:, 0, :KC], mu[:], mu[:], ALU.mult)
                    nc.vector.tensor_tensor(var[:], var[:], scr[:, 0, :KC], ALU.subtract)
                    nc.vector.tensor_scalar(var[:], var[:], EPS, None, op0=ALU.add)
                    istd = mlp_sb.tile([128, KC], F32, tag="istd")
                    istd_s = mlp_sb.tile([128, KC], F32, tag="istds")
                    _rsqrt(nc, mlp_sb, var[:], istd[:], KC, "rsm")
                    nc.vector.tensor_scalar(istd_s[:], istd[:], SQ, None,
                                            op0=ALU.mult)
                    for r in range(KC):
                        # PV keys (unscaled)
                        nc.vector.scalar_tensor_tensor(
                            out=kv8[:, h, r, :128], in0=m_f[:, r, :],
                            scalar=mu[:, r:r + 1],
                            in1=istd[:, r:r + 1].broadcast_to([128, DB]),
                            op0=ALU.subtract, op1=ALU.mult)
                        # QK keys (SQ-scaled), packed layout
                        nc.vector.scalar_tensor_tensor(
                            out=kvs8[:, r, h, :], in0=m_f[:, r, :],
                            scalar=mu[:, r:r + 1],
                            in1=istd_s[:, r:r + 1].broadcast_to([128, DB]),
                            op0=ALU.subtract, op1=ALU.mult)
                # kT8[p=(h,m), r, proto, j] = kvs8[proto, r, h, 2m+j]
                nc.sync.dma_start_transpose(
                    kT8[:].bitcast(BF16).rearrange("p r t one -> p r (t one)"),
                    kvs8[:].bitcast(BF16).rearrange("p r h u -> p (r h u)"))
                nc.vector.tensor_copy(kT8u[:],
                                      kT8[:].rearrange("p r t j -> p r j t"))

            # ---- phase 1: attention ----
            out_v = out_d  # [bt, p, s, h, d]

            def emit_bn(b, subs):
                st = state[b]
                qr_v = st["qr"][:].rearrange("p s (h d) -> p s h d", h=HPC)
                for sub in subs:
                    for h in range(HPC):
                        nc.vector.bn_stats(st["st6"][:, sub, h], qr_v[:, sub, h])

            def emit_stats_alloc(b):
                state[b]["st6"] = stat_p.tile([128, nsub, HPC, 6], F32,
                                              tag="st6", name=f"st6{b}")

            def emit_istd(b):
                st = state[b]
                st6 = st["st6"]
                t1 = stat_p.tile([128, nsub, HPC, 1], F32, tag="t1")
                t2 = stat_p.tile([128, nsub, HPC, 1], F32, tag="t2")
                vv = stat_p.tile([128, nsub, HPC], F32, tag="vv")
                istd_t = stat_p.tile([128, nsub, HPC], F32, tag="istd",
                                     name=f"istd{b}")
                # var = (M2e + M2o + 32*(me-mo)^2)/128 ; fold the SQ^2 = DB^-0.5
                # score scale in as *sqrt(DB), plus eps
                nc.vector.tensor_tensor(t1[:], st6[:, :, :, 2:3],
                                        st6[:, :, :, 5:6], ALU.add)
                nc.vector.tensor_tensor(t2[:], st6[:, :, :, 1:2],
                                        st6[:, :, :, 4:5], ALU.subtract)
                nc.vector.tensor_tensor(t2[:], t2[:], t2[:], ALU.mult)
                nc.vector.scalar_tensor_tensor(
                    out=t1[:], in0=t2[:], scalar=32.0, in1=t1[:],
                    op0=ALU.mult, op1=ALU.add)
                sdb = float(DB) ** 0.5
                nc.vector.tensor_scalar(vv[:], t1[:, :, :, 0], sdb / DB,
                                        EPS * sdb, op0=ALU.mult, op1=ALU.add)
                _rsqrt(nc, stat_p, vv[:].rearrange("p s h -> p (s h)"),
                       istd_t[:].rearrange("p s h -> p (s h)"),
                       nsub * HPC, "rsq")
                st["istd"] = istd_t

            def emit_qb(b, t):
                st = state[b]
                qb = qb_p.tile([128, 4, HPC, DB], FP8E4, tag="qb",
                               name=f"qb{b}_{t}")
                nc.gpsimd.tensor_tensor(
                    qb[:],
                    st["qr"][:, 4 * t:4 * t + 4, :].rearrange(
                        "p s (h d) -> p s h d", h=HPC),
                    st["istd"][:, 4 * t:4 * t + 4, :, None].broadcast_to(
                        [128, 4, HPC, DB]),
                    ALU.mult)
                qT8 = qT_p.tile([128, 4, 128, 2], FP8E4, tag="qT",
                                name=f"qT{b}_{t}")
                nc.sync.dma_start_transpose(
                    qT8[:].bitcast(BF16).rearrange("p s t one -> p s (t one)"),
                    qb[:].bitcast(BF16).rearrange("p s h u -> p (s h u)"))
                st.setdefault("qT", {})[t] = qT8

            def emit_front(b, t, h, ps_pool):
                qT8 = state[b]["qT"][t]
                pss = ps_pool.tile([128, KC, 512], F32, tag="ps",
                                   name=f"sc{b}_{t}_{h}")
                qrhs = qT8[64 * h:64 * h + 64].rearrange("p s t j -> p j s t")
                for c in range(KC):
                    nc.tensor.matmul(
                        pss[:, c, :],
                        kT8u[64 * h:64 * h + 64, c],
                        qrhs, start=True, stop=True, perf_mode=DR)
                ee = e_p.tile([128, 4, KC, 128], FP8E5, tag="ee",
                              name=f"ee{b}_{t}_{h}")
                nc.scalar.activation(
                    ee[:].rearrange("p s c t -> p c s t"),
                    pss[:].rearrange("p c (s t) -> p c s t", s=4),
                    ACT.Exp, bias=ebias[:, 0:1])
                return (b, t, h, pss, ee)

            def emit_back(item, obs):
                b, t, h, pss, ee = item
                if h == 0:
                    obs[(b, t)] = ob_p.tile([128, 4, HPC, DB + 1], BF16,
                                            tag="ob", name=f"ob{b}_{t}")
                ob = obs[(b, t)]
                for s in range(4):
                    for i in range(2):
                        nc.tensor.matmul(
                            pss[:, s, :129],
                            ee[:, s, 2 * i:2 * i + 2, :],
                            kv8[:, h, 2 * i:2 * i + 2, :129],
                            start=(i == 0), stop=(i == 1), perf_mode=DR)
                nc.vector.tensor_copy(ob[:, :, h, :], pss[:, :, :129])
                if h == HPC - 1:
                    nc.sync.dma_start(out_v[b * nt + t], ob[:])
                    del obs[(b, t)]

            with tc.tile_pool(name="mps", bufs=2, space="PSUM") as ps_p:
                # 3-deep batch pipeline: loads(b+3) during b, bn(b+2) during
                # b (chunks after each drain), istd(b+2) at (b, nt-1),
                # qb/transpose(b+1) one per step.  Prologue primes loads(0..2)
                # + stats(0,1) + qb(0, all).
                if nt >= 8:
                    emit_stats_alloc(0)
                    emit_bn(0, range(nsub))
                    emit_istd(0)
                    if nbb > 1:
                        emit_stats_alloc(1)
                        emit_bn(1, range(nsub))
                        emit_istd(1)
                    for t0 in range(nt):
                        emit_qb(0, t0)
                    if nbb > 2:
                        emit_loads(2)

                    # bn(b+2) sub-chunks over half-steps (t,h): t in 2..7,
                    # skipping the last h-slot (istd goes there)
                    bn_slots = [(t, h) for t in range(2, nt) for h in range(HPC)]
                    bn_sched = {}
                    per = (nsub + len(bn_slots) - 1) // len(bn_slots)
                    i = 0
                    for slot in bn_slots:
                        bn_sched[slot] = range(i, min(i + per, nsub))
                        i += per
                        if i >= nsub:
                            break
                    load_sched = {t: (t, t + 1) for t in range(8)}

                    obs = {}
                    prev = None
                    for b in range(nbb):
                        for t in range(nt):
                            if b + 1 < nbb:
                                emit_qb(b + 1, t)
                            if b + 3 < nbb and t in load_sched:
                                lo, hi = load_sched[t]
                                emit_loads(b + 3, lo, hi)
                            if b + 2 < nbb and t == 2:
                                emit_stats_alloc(b + 2)
                            for h in range(HPC):
                                if b + 2 < nbb and (t, h) in bn_sched:
                                    emit_bn(b + 2, bn_sched[(t, h)])
                                if (b + 2 < nbb and t == nt - 1
                                        and h == HPC - 1):
                                    emit_istd(b + 2)
                                cur = emit_front(b, t, h, ps_p)
                                if prev is not None:
                                    emit_back(prev, obs)
                                prev = cur
                    emit_back(prev, obs)
                else:
                    # small-config correctness mode: serial prep per batch
                    emit_stats_alloc(0)
                    emit_bn(0, range(nsub))
                    emit_istd(0)
                    for t0 in range(min(2, nt)):
                        emit_qb(0, t0)
                    obs = {}
                    prev = None
                    for b in range(nbb):
                        for t in range(nt):
                            if t + 2 < nt:
                                emit_qb(b, t + 2)
                            if b + 1 < nbb and t == 0:
                                if b + 2 < nbb:
                                    emit_loads(b + 2)
                                emit_stats_alloc(b + 1)
                                emit_bn(b + 1, range(nsub))
                                emit_istd(b + 1)
                                for t0 in range(min(2, nt)):
                                    emit_qb(b + 1, t0)
                            for h in range(HPC):
                                cur = emit_front(b, t, h, ps_p)
                                if prev is not None:
                                    emit_back(prev, obs)
                                prev = cur
                    emit_back(prev, obs)
    nc.compile()
    return nc


_CACHE = {}


def _get_nc(nbb, nt):
    key = (nbb, nt)
    if key not in _CACHE:
        _CACHE[key] = build_nc(nbb, nt)
    return _CACHE[key]


def make_in_maps(queries, mem_params, w1, b1, w2, b2, w3, b3, w4, b4):
    f = np.float32
    shared = {
        "w1t": np.ascontiguousarray(np.asarray(w1.T, f).astype(bfloat16)),
        "w2t": np.ascontiguousarray(np.asarray(w2.T, f).astype(bfloat16)),
        "w3t": np.ascontiguousarray(np.asarray(w3.T, f).astype(bfloat16)),
        "w4t": np.ascontiguousarray(np.asarray(w4.T, f).astype(bfloat16)),
        "b1r": np.ascontiguousarray(np.asarray(b1, f).reshape(JC, DB).T),
        "b2r": np.ascontiguousarray(np.asarray(b2, f).reshape(JC, DB).T),
        "b3r": np.ascontiguousarray(np.asarray(b3, f).reshape(JC, DB).T),
        "b4rep": np.ascontiguousarray(np.tile(np.asarray(b4, f), (DB, 1))),
    }
    in_maps = []
    for c in range(NCORES):
        mp = np.asarray(mem_params, f)[0, :, c * HPC:(c + 1) * HPC, :]  # [P,HPC,DB]
        m = dict(shared)
        m["q"] = np.ascontiguousarray(
            queries[:, :, c * DS:(c + 1) * DS], dtype=f).reshape(-1, DS)
        m["mpt"] = np.ascontiguousarray(mp.transpose(1, 2, 0))  # [HPC, DB, P]
        in_maps.append(m)
    return in_maps


def assemble(res, nbb, ntok):
    """Per-core tile dumps [bt, p, s, h, d+den] bf16 -> full [B, N, D] f32.
    Host divides by the denominator column (device skips the normalize)."""
    nt = ntok // 512
    parts = []
    for c in range(NCORES):
        a = np.asarray(res[c]["out"]).astype(np.float32)
        a = a.reshape(nbb, nt, 128, 4, HPC, DB + 1).transpose(0, 1, 3, 2, 4, 5)
        a = a[..., :DB] / a[..., DB:]
        parts.append(np.ascontiguousarray(a).reshape(nbb, ntok, DS))
    return np.concatenate(parts, axis=-1)


def kernel(queries, mem_params, w1, b1, w2, b2, w3, b3, w4, b4):
    queries = np.asarray(queries, np.float32)
    nbb, ntok, dd = queries.shape
    nt = ntok // 512
    nc = _get_nc(nbb, nt)
    in_maps = make_in_maps(queries, mem_params, w1, b1, w2, b2, w3, b3, w4, b4)
    res = run_bass_kernel_spmd(nc, in_maps, list(range(NCORES))).results
    return np.ascontiguousarray(assemble(res, nbb, ntok))


if __name__ == "__main__":
    nc = build_nc(1, 1)
    print("built ok")


# revision 3
# speedup vs baseline: 1.0182x; 1.0182x over previous
"""Trainium2 Bass kernel for BlockPrototypeMemory (sparse block attention), v2.

Reference computation (fp32):
  mem = 4-layer MLP(mem_params)            [1, P, NB, DB]
  mem = block_ln(mem); q = block_ln(queries)
  scores = einsum('bnhd,zmhd->bhnm', q*DB^-.5, mem)
  out = softmax(scores) @ mem              [B, N, D]

Sharding: tensor-parallel over the NB=16 blocks; each of the 8 cores owns
2 blocks (a 256-wide slice of D).  No collectives: the host reassembles.

v2 design (vs the 638us baseline, which stalled ~50% on PSUM-drain deps):
 - QK^T in fp8e4 DoubleRow (2 k-tiles of 64 along DB, even/odd d pairs
   packed in bytes so the 2-byte DMA transpose carries fp8 pairs).
   sqrt(DB^-0.5) is folded into each side (q via istd, k at cast).
 - exp on ACT straight out of PSUM -> E in fp8e5 (bias -1 folded into the
   softmax ratio); PV in DoubleRow fp8 (E e5m2 x kv e4m3) with a ones
   column accumulating the denominator.
 - PV accumulates into the *same* PSUM tile the scores came from (banks are
   dead after exp), so two 4-bank tiles give a 2-deep pipeline and the PE
   never waits on PSUM for more than one drain.
 - q LN stats via per-(subtile, block) single-group bn_stats on DVE
   (even/odd halves recombined with 4 vector ops); mean subtraction of q
   is dropped (keys are zero-mean).
 - Output stored bf16 in tile-dump layout [bt, p, s, h, d]; host transposes
   and converts (device normalizes by the denominator; host only relayouts).
Emission order per step interleaves next-batch loads/stats so the ACT
engine (exp, ~1.85us/unit) streams with no batch-boundary bubble.
"""

import numpy as np
from ml_dtypes import bfloat16

import concourse.bass as bass
import concourse.bacc as bacc
import concourse.mybir as mybir
import concourse.tile as tile
from concourse.bass import ts
from concourse.bass_utils import run_bass_kernel_spmd

F32 = mybir.dt.float32
BF16 = mybir.dt.bfloat16
FP8E4 = mybir.dt.float8e4
FP8E5 = mybir.dt.float8e5
I32 = mybir.dt.int32
ALU = mybir.AluOpType
ACT = mybir.ActivationFunctionType
AX = mybir.AxisListType
DR = mybir.MatmulPerfMode.DoubleRow

# ---- problem dims ----
B, N, D = 8, 4096, 2048
NB, DB = 16, 128
P = 512            # prototypes (attention keys)
HID = 4 * DB       # 512 MLP hidden
NCORES = 8
HPC = NB // NCORES  # 2 blocks per core
DS = HPC * DB       # 256 per-core D slice
EPS = 1e-5
KC = P // 128       # 4 key chunks
JC = HID // 128     # 4 hidden chunks
SQ = float(DB) ** -0.25   # sqrt of the DB^-0.5 score scale, one per side
EBIAS = -1.0              # exp(s + EBIAS): cancels in the softmax ratio


def _rsqrt(nc, pool, x, out, F, tag):
    """out = 1/sqrt(x) elementwise on DVE only (no ACT table swaps)."""
    ti = pool.tile([128, F], I32, tag=tag + "_i")
    t2 = pool.tile([128, F], F32, tag=tag + "_t")
    y = pool.tile([128, F], F32, tag=tag + "_y")
    magic = pool.tile([128, 1], I32, tag=tag + "_m")
    nc.vector.memset(magic[:], 0x5F3759DF)
    nc.vector.tensor_scalar(ti[:], x.bitcast(I32), 1, None, op0=ALU.arith_shift_right)
    nc.vector.tensor_tensor(ti[:], magic[:].broadcast_to([128, F]), ti[:], ALU.subtract)
    yv = ti[:].bitcast(F32)
    for it in range(3):
        dst = out if it == 2 else y[:]
        nc.vector.tensor_tensor(t2[:], yv, yv, ALU.mult)
        nc.vector.tensor_tensor(t2[:], t2[:], x, ALU.mult)
        nc.vector.tensor_scalar(t2[:], t2[:], -0.5, 1.5, op0=ALU.mult, op1=ALU.add)
        nc.vector.tensor_tensor(dst, yv, t2[:], ALU.mult)
        yv = y[:]


def build_nc(nbb=B, nt=N // 512):
    """Per-core Bass module. nbb batches x nt 512-token groups."""
    nc = bacc.Bacc("TRN2", target_bir_lowering=False, debug=False)
    rows = nbb * nt * 512
    nsub = 4 * nt           # 128-token subtiles per batch

    q_d = nc.dram_tensor("q", [rows, DS], F32, kind="ExternalInput").ap()
    mpt_d = nc.dram_tensor("mpt", [HPC, DB, P], F32, kind="ExternalInput").ap()
    w1t_d = nc.dram_tensor("w1t", [DB, HID], BF16, kind="ExternalInput").ap()
    w2t_d = nc.dram_tensor("w2t", [HID, HID], BF16, kind="ExternalInput").ap()
    w3t_d = nc.dram_tensor("w3t", [HID, HID], BF16, kind="ExternalInput").ap()
    w4t_d = nc.dram_tensor("w4t", [HID, DB], BF16, kind="ExternalInput").ap()
    b1r_d = nc.dram_tensor("b1r", [DB, JC], F32, kind="ExternalInput").ap()
    b2r_d = nc.dram_tensor("b2r", [DB, JC], F32, kind="ExternalInput").ap()
    b3r_d = nc.dram_tensor("b3r", [DB, JC], F32, kind="ExternalInput").ap()
    b4rep_d = nc.dram_tensor("b4rep", [DB, DB], F32, kind="ExternalInput").ap()
    # tile-dump layout: [b*nt + t, p, s, h, d+den] bf16; host reassembles
    out_d = nc.dram_tensor("out", [nbb * nt, 128, 4, HPC, DB + 1], BF16,
                           kind="ExternalOutput").ap()

    q_v = q_d.rearrange("(b s p) d -> b p s d", b=nbb, p=128)

    with tile.TileContext(nc) as tc:
        with (
            tc.tile_pool(name="const", bufs=1) as const,
            tc.tile_pool(name="qres", bufs=3) as qres_p,
            tc.tile_pool(name="stat", bufs=3) as stat_p,
            tc.tile_pool(name="qb", bufs=12) as qb_p,
            tc.tile_pool(name="qT", bufs=10) as qT_p,
            tc.tile_pool(name="E", bufs=6) as e_p,
            tc.tile_pool(name="ob", bufs=8) as ob_p,
        ):
            # ---- persistent small tensors ----
            ebias = const.tile([128, 1], F32)
            nc.vector.memset(ebias[:], EBIAS)
            # PV keys: LN'd mem (unscaled) fp8e4 + ones col for denominator
            kv8 = const.tile([128, HPC, KC, 130], FP8E4)
            nc.vector.memset(kv8[:, :, :, 128:130], 0.0)
            nc.vector.memset(kv8[:, :, :, 128:129], 1.0)
            # QK keys: LN'd mem * SQ, transposed+packed (even/odd d pairs)
            kT8 = const.tile([128, KC, 128, 2], FP8E4)
            # j-major copy: dual-fp8 Ldweights requires contiguous k-tiles
            kT8u = const.tile([128, KC, 2, 128], FP8E4)

            # ---- batch-0 loads first: DMA runs during the MLP phase ----
            state = {}

            def emit_loads(b, lo=0, hi=None):
                nchunk = min(8, nsub)
                if lo == 0:
                    state[b] = {"qr": qres_p.tile([128, nsub, DS], F32,
                                                  tag="qr", name=f"qr{b}")}
                qr = state[b]["qr"]
                w = nsub // nchunk
                if hi is None:
                    hi = nchunk
                for k in range(lo, hi):
                    nc.sync.dma_start(qr[:, k * w:(k + 1) * w, :],
                                      q_v[b, :, k * w:(k + 1) * w, :])

            emit_loads(0)
            if nbb > 1:
                emit_loads(1)

            # ---- phase 0: mem MLP + LN per block ----
            with (
                tc.tile_pool(name="mlp_ps", bufs=2, space="PSUM") as mlp_ps,
                tc.tile_pool(name="mlp_ps4", bufs=2, space="PSUM") as mlp_ps4,
                tc.tile_pool(name="mlp_sb", bufs=1) as mlp_sb,
            ):
                b1r = const.tile([DB, JC], F32)
                b2r = const.tile([DB, JC], F32)
                b3r = const.tile([DB, JC], F32)
                b4rep = const.tile([DB, DB], F32)
                w1t_b = mlp_sb.tile([DB, HID], BF16, tag="w1b")
                w2t_b = mlp_sb.tile([128, JC, HID], BF16, tag="w2b")
                w3t_b = mlp_sb.tile([128, JC, HID], BF16, tag="w3b")
                w4t_b = mlp_sb.tile([128, JC, DB], BF16, tag="w4b")
                nc.sync.dma_start(w1t_b[:], w1t_d)
                nc.sync.dma_start(w2t_b[:], w2t_d.rearrange("(c p) o -> p c o", p=128))
                nc.sync.dma_start(w3t_b[:], w3t_d.rearrange("(c p) o -> p c o", p=128))
                nc.sync.dma_start(w4t_b[:], w4t_d.rearrange("(c p) o -> p c o", p=128))
                nc.sync.dma_start(b1r[:], b1r_d)
                nc.sync.dma_start(b2r[:], b2r_d)
                nc.sync.dma_start(b3r[:], b3r_d)
                nc.sync.dma_start(b4rep[:], b4rep_d)

                # packed pre-transpose keys: [p(protos), r, h, d] fp8e4
                kvs8 = mlp_sb.tile([128, KC, HPC, DB], FP8E4, tag="kvs8")
                for h in range(HPC):
                    x_f = mlp_sb.tile([DB, P], F32, tag="xf")
                    x_b = mlp_sb.tile([DB, P], BF16, tag="xb")
                    nc.sync.dma_start(x_f[:], mpt_d[h])
                    nc.vector.tensor_copy(x_b[:], x_f[:])
                    h1 = mlp_sb.tile([128, JC, P], BF16, tag="h1")
                    for j in range(JC):
                        ps = mlp_ps.tile([128, P], F32, tag="ps")
                        nc.tensor.matmul(ps[:], w1t_b[:, ts(j, 128)], x_b[:],
                                         start=True, stop=True)
                        nc.scalar.activation(h1[:, j, :], ps[:], ACT.Relu,
                                             bias=b1r[:, j:j + 1])
                    h2 = mlp_sb.tile([128, JC, P], BF16, tag="h2")
                    for j in range(JC):
                        ps = mlp_ps.tile([128, P], F32, tag="ps")
                        for i in range(JC):
                            nc.tensor.matmul(ps[:], w2t_b[:, i, ts(j, 128)],
                                             h1[:, i, :],
                                             start=(i == 0), stop=(i == JC - 1))
                        nc.scalar.activation(h2[:, j, :], ps[:], ACT.Relu,
                                             bias=b2r[:, j:j + 1])
                    h3 = mlp_sb.tile([128, JC, P], BF16, tag="h3")
                    for j in range(JC):
                        ps = mlp_ps.tile([128, P], F32, tag="ps")
                        for i in range(JC):
                            nc.tensor.matmul(ps[:], w3t_b[:, i, ts(j, 128)],
                                             h2[:, i, :],
                                             start=(i == 0), stop=(i == JC - 1))
                        nc.scalar.activation(h3[:, j, :], ps[:], ACT.Relu,
                                             bias=b3r[:, j:j + 1])
                    # L4 in row layout [keys, DB] so LN stats are free-dim
                    m_f = mlp_sb.tile([128, KC, DB], F32, tag="mf")
                    for r in range(KC):
                        ps4 = mlp_ps4.tile([128, DB], F32, tag="ps4")
                        for i in range(JC):
                            nc.tensor.matmul(ps4[:], h3[:, i, ts(r, 128)],
                                             w4t_b[:, i, :],
                                             start=(i == 0), stop=(i == JC - 1))
                        nc.vector.tensor_tensor(m_f[:, r, :], ps4[:], b4rep[:],
                                                ALU.add)
                    # block-LN over DB (free dim) for the 4 row chunks
                    msum = mlp_sb.tile([128, KC], F32, tag="msum")
                    msq = mlp_sb.tile([128, KC], F32, tag="msq")
                    scr = mlp_sb.tile([128, KC, DB], F32, tag="scr")
                    nc.vector.reduce_sum(msum[:], m_f[:], axis=AX.X)
                    nc.scalar.activation(scr[:], m_f[:], ACT.Square)
                    nc.vector.reduce_sum(msq[:], scr[:], axis=AX.X)
                    mu = mlp_sb.tile([128, KC], F32, tag="mu")
                    var = mlp_sb.tile([128, KC], F32, tag="var")
                    nc.vector.tensor_scalar(mu[:], msum[:], 1.0 / DB, None, op0=ALU.mult)
                    nc.vector.tensor_scalar(var[:], msq[:], 1.0 / DB, None, op0=ALU.mult)
                    nc.vector.tensor_tensor(scr[:, 0, :KC], mu[:], mu[:], ALU.mult)
                    nc.vector.tensor_tensor(var[:], var[:], scr[:, 0, :KC], ALU.subtract)
                    nc.vector.tensor_scalar(var[:], var[:], EPS, None, op0=ALU.add)
                    istd = mlp_sb.tile([128, KC], F32, tag="istd")
                    istd_s = mlp_sb.tile([128, KC], F32, tag="istds")
                    _rsqrt(nc, mlp_sb, var[:], istd[:], KC, "rsm")
                    nc.vector.tensor_scalar(istd_s[:], istd[:], SQ, None,
                                            op0=ALU.mult)
                    for r in range(KC):
                        # PV keys (unscaled)
                        nc.vector.scalar_tensor_tensor(
                            out=kv8[:, h, r, :128], in0=m_f[:, r, :],
                            scalar=mu[:, r:r + 1],
                            in1=istd[:, r:r + 1].broadcast_to([128, DB]),
                            op0=ALU.subtract, op1=ALU.mult)
                        # QK keys (SQ-scaled), packed layout
                        nc.vector.scalar_tensor_tensor(
                            out=kvs8[:, r, h, :], in0=m_f[:, r, :],
                            scalar=mu[:, r:r + 1],
                            in1=istd_s[:, r:r + 1].broadcast_to([128, DB]),
                            op0=ALU.subtract, op1=ALU.mult)
                # kT8[p=(h,m), r, proto, j] = kvs8[proto, r, h, 2m+j]
                nc.sync.dma_start_transpose(
                    kT8[:].bitcast(BF16).rearrange("p r t one -> p r (t one)"),
                    kvs8[:].bitcast(BF16).rearrange("p r h u -> p (r h u)"))
                nc.vector.tensor_copy(kT8u[:],
                                      kT8[:].rearrange("p r t j -> p r j t"))

            # ---- phase 1: attention ----
            out_v = out_d  # [bt, p, s, h, d]

            def emit_bn(b, subs):
                st = state[b]
                qr_v = st["qr"][:].rearrange("p s (h d) -> p s h d", h=HPC)
                for sub in subs:
                    for h in range(HPC):
                        nc.vector.bn_stats(st["st6"][:, sub, h], qr_v[:, sub, h])

            def emit_stats_alloc(b):
                state[b]["st6"] = stat_p.tile([128, nsub, HPC, 6], F32,
                                              tag="st6", name=f"st6{b}")

            def emit_istd(b):
                st = state[b]
                st6 = st["st6"]
                t1 = stat_p.tile([128, nsub, HPC, 1], F32, tag="t1")
                t2 = stat_p.tile([128, nsub, HPC, 1], F32, tag="t2")
                vv = stat_p.tile([128, nsub, HPC], F32, tag="vv")
                istd_t = stat_p.tile([128, nsub, HPC], F32, tag="istd",
                                     name=f"istd{b}")
                # var = (M2e + M2o + 32*(me-mo)^2)/128 ; fold the SQ^2 = DB^-0.5
                # score scale in as *sqrt(DB), plus eps
                nc.vector.tensor_tensor(t1[:], st6[:, :, :, 2:3],
                                        st6[:, :, :, 5:6], ALU.add)
                nc.vector.tensor_tensor(t2[:], st6[:, :, :, 1:2],
                                        st6[:, :, :, 4:5], ALU.subtract)
                nc.vector.tensor_tensor(t2[:], t2[:], t2[:], ALU.mult)
                nc.vector.scalar_tensor_tensor(
                    out=t1[:], in0=t2[:], scalar=32.0, in1=t1[:],
                    op0=ALU.mult, op1=ALU.add)
                sdb = float(DB) ** 0.5
                nc.vector.tensor_scalar(vv[:], t1[:, :, :, 0], sdb / DB,
                                        EPS * sdb, op0=ALU.mult, op1=ALU.add)
                _rsqrt(nc, stat_p, vv[:].rearrange("p s h -> p (s h)"),
                       istd_t[:].rearrange("p s h -> p (s h)"),
                       nsub * HPC, "rsq")
                st["istd"] = istd_t

            def emit_qb(b, t):
                st = state[b]
                qb = qb_p.tile([128, 4, HPC, DB], FP8E4, tag="qb",
                               name=f"qb{b}_{t}")
                nc.gpsimd.tensor_tensor(
                    qb[:],
                    st["qr"][:, 4 * t:4 * t + 4, :].rearrange(
                        "p s (h d) -> p s h d", h=HPC),
                    st["istd"][:, 4 * t:4 * t + 4, :, None].broadcast_to(
                        [128, 4, HPC, DB]),
                    ALU.mult)
                qT8 = qT_p.tile([128, 4, 128, 2], FP8E4, tag="qT",
                                name=f"qT{b}_{t}")
                nc.sync.dma_start_transpose(
                    qT8[:].bitcast(BF16).rearrange("p s t one -> p s (t one)"),
                    qb[:].bitcast(BF16).rearrange("p s h u -> p (s h u)"))
                st.setdefault("qT", {})[t] = qT8

            def emit_front(b, t, h, ps_pool):
                qT8 = state[b]["qT"][t]
                pss = ps_pool.tile([128, KC, 512], F32, tag="ps",
                                   name=f"sc{b}_{t}_{h}")
                qrhs = qT8[64 * h:64 * h + 64].rearrange("p s t j -> p j s t")
                for c in range(KC):
                    nc.tensor.matmul(
                        pss[:, c, :],
                        kT8u[64 * h:64 * h + 64, c],
                        qrhs, start=True, stop=True, perf_mode=DR)
                ee = e_p.tile([128, 4, KC, 128], FP8E5, tag="ee",
                              name=f"ee{b}_{t}_{h}")
                nc.scalar.activation(
                    ee[:].rearrange("p s c t -> p c s t"),
                    pss[:].rearrange("p c (s t) -> p c s t", s=4),
                    ACT.Exp, bias=ebias[:, 0:1])
                return (b, t, h, pss, ee)

            def emit_back(item, obs):
                b, t, h, pss, ee = item
                if h == 0:
                    obs[(b, t)] = ob_p.tile([128, 4, HPC, DB + 1], BF16,
                                            tag="ob", name=f"ob{b}_{t}")
                ob = obs[(b, t)]
                for s in range(4):
                    for i in range(2):
                        nc.tensor.matmul(
                            pss[:, s, :129],
                            ee[:, s, 2 * i:2 * i + 2, :],
                            kv8[:, h, 2 * i:2 * i + 2, :129],
                            start=(i == 0), stop=(i == 1), perf_mode=DR)
                nc.vector.tensor_copy(ob[:, :, h, :], pss[:, :, :129])
                if h == HPC - 1:
                    nc.sync.dma_start(out_v[b * nt + t], ob[:])
                    del obs[(b, t)]

            with tc.tile_pool(name="mps", bufs=2, space="PSUM") as ps_p:
                # 3-deep batch pipeline: loads(b+3) during b, bn(b+2) during
                # b (chunks after each drain), istd(b+2) at (b, nt-1),
                # qb/transpose(b+1) one per step.  Prologue primes loads(0..2)
                # + stats(0,1) + qb(0, all).
                if nt >= 8:
                    emit_stats_alloc(0)
                    emit_bn(0, range(nsub))
                    emit_istd(0)
                    if nbb > 1:
                        emit_stats_alloc(1)
                        emit_bn(1, range(nsub))
                        emit_istd(1)
                    for t0 in range(nt):
                        emit_qb(0, t0)
                    if nbb > 2:
                        emit_loads(2)

                    # bn(b+2) sub-chunks over half-steps (t,h): t in 2..7,
                    # skipping the last h-slot (istd goes there)
                    bn_slots = [(t, h) for t in range(2, nt) for h in range(HPC)]
                    bn_sched = {}
                    per = (nsub + len(bn_slots) - 1) // len(bn_slots)
                    i = 0
                    for slot in bn_slots:
                        bn_sched[slot] = range(i, min(i + per, nsub))
                        i += per
                        if i >= nsub:
                            break
                    load_sched = {t: (t, t + 1) for t in range(8)}

                    obs = {}
                    prev = None
                    for b in range(nbb):
                        for t in range(nt):
                            if b + 1 < nbb:
                                emit_qb(b + 1, t)
                            if b + 3 < nbb and t in load_sched:
                                lo, hi = load_sched[t]
                                emit_loads(b + 3, lo, hi)
                            if b + 2 < nbb and t == 2:
                                emit_stats_alloc(b + 2)
                            for h in range(HPC):
                                if b + 2 < nbb and (t, h) in bn_sched:
                                    emit_bn(b + 2, bn_sched[(t, h)])
                                if (b + 2 < nbb and t == nt - 1
                                        and h == HPC - 1):
                                    emit_istd(b + 2)
                                cur = emit_front(b, t, h, ps_p)
                                if prev is not None:
                                    emit_back(prev, obs)
                                prev = cur
                    emit_back(prev, obs)
                else:
                    # small-config correctness mode: serial prep per batch
                    emit_stats_alloc(0)
                    emit_bn(0, range(nsub))
                    emit_istd(0)
                    for t0 in range(min(2, nt)):
                        emit_qb(0, t0)
                    obs = {}
                    prev = None
                    for b in range(nbb):
                        for t in range(nt):
                            if t + 2 < nt:
                                emit_qb(b, t + 2)
                            if b + 1 < nbb and t == 0:
                                if b + 2 < nbb:
                                    emit_loads(b + 2)
                                emit_stats_alloc(b + 1)
                                emit_bn(b + 1, range(nsub))
                                emit_istd(b + 1)
                                for t0 in range(min(2, nt)):
                                    emit_qb(b + 1, t0)
                            for h in range(HPC):
                                cur = emit_front(b, t, h, ps_p)
                                if prev is not None:
                                    emit_back(prev, obs)
                                prev = cur
                    emit_back(prev, obs)
    nc.compile()
    return nc


_CACHE = {}


def _get_nc(nbb, nt):
    key = (nbb, nt)
    if key not in _CACHE:
        _CACHE[key] = build_nc(nbb, nt)
    return _CACHE[key]


def make_in_maps(queries, mem_params, w1, b1, w2, b2, w3, b3, w4, b4):
    f = np.float32
    shared = {
        "w1t": np.ascontiguousarray(np.asarray(w1.T, f).astype(bfloat16)),
        "w2t": np.ascontiguousarray(np.asarray(w2.T, f).astype(bfloat16)),
        "w3t": np.ascontiguousarray(np.asarray(w3.T, f).astype(bfloat16)),
        "w4t": np.ascontiguousarray(np.asarray(w4.T, f).astype(bfloat16)),
        "b1r": np.ascontiguousarray(np.asarray(b1, f).reshape(JC, DB).T),
        "b2r": np.ascontiguousarray(np.asarray(b2, f).reshape(JC, DB).T),
        "b3r": np.ascontiguousarray(np.asarray(b3, f).reshape(JC, DB).T),
        "b4rep": np.ascontiguousarray(np.tile(np.asarray(b4, f), (DB, 1))),
    }
    in_maps = []
    for c in range(NCORES):
        mp = np.asarray(mem_params, f)[0, :, c * HPC:(c + 1) * HPC, :]  # [P,HPC,DB]
        m = dict(shared)
        m["q"] = np.ascontiguousarray(
            queries[:, :, c * DS:(c + 1) * DS], dtype=f).reshape(-1, DS)
        m["mpt"] = np.ascontiguousarray(mp.transpose(1, 2, 0))  # [HPC, DB, P]
        in_maps.append(m)
    return in_maps


def assemble(res, nbb, ntok):
    """Per-core tile dumps [bt, p, s, h, d+den] bf16 -> full [B, N, D] f32.
    Host divides by the denominator column (device skips the normalize)."""
    nt = ntok // 512
    parts = []
    for c in range(NCORES):
        a = np.asarray(res[c]["out"]).astype(np.float32)
        a = a.reshape(nbb, nt, 128, 4, HPC, DB + 1).transpose(0, 1, 3, 2, 4, 5)
        a = a[..., :DB] / a[..., DB:]
        parts.append(np.ascontiguousarray(a).reshape(nbb, ntok, DS))
    return np.concatenate(parts, axis=-1)


def kernel(queries, mem_params, w1, b1, w2, b2, w3, b3, w4, b4):
    queries = np.asarray(queries, np.float32)
    nbb, ntok, dd = queries.shape
    nt = ntok // 512
    nc = _get_nc(nbb, nt)
    in_maps = make_in_maps(queries, mem_params, w1, b1, w2, b2, w3, b3, w4, b4)
    res = run_bass_kernel_spmd(nc, in_maps, list(range(NCORES))).results
    return np.ascontiguousarray(assemble(res, nbb, ntok))


if __name__ == "__main__":
    nc = build_nc(1, 1)
    print("built ok")
